# revision 1
# baseline (speedup 1.0000x reference)
"""Trainium2 Bass kernel for batched displacement-operator construction.

Math: Da[b] = P_b o (C_b - i S_b) where C = V diag(cos r lam) V^T,
S = V diag(sin r lam) V^T and P is the unit-modulus Toeplitz phase
matrix w^(i-j), w = i*alpha/|alpha|.

Key structure: the generator a+a^dag anticommutes with parity
Pi = diag((-1)^j), so eigenpairs come in (lam, -lam) pairs with
v_{-lam} = +-Pi v_lam.  Hence C is nonzero only at even i+j and S only
at odd i+j (checkerboard), and both are determined by the 512 negative-
lambda columns and the even/odd row halves A = V[0::2, :512],
B = V[1::2, :512]:

    C_ee = A diag(2 cos) A^T   C_oo = B diag(2 cos) B^T
    S_eo = A diag(2 sin) B^T   S_oe = S_eo^T

The device computes the three 512x512 blocks in bf16 (4.7x less PE work
than the dense 2x1024^3 formulation); C_ee/C_oo being symmetric, only
their upper-triangle 128-blocks are matmul'd and shipped (ragged slabs,
another ~28% off the C matmuls and ~27% off the output bytes).  The
host mirrors the lower-triangle C blocks, applies the rank-1 Toeplitz
phase (u_i v_j outer products), uses S symmetry for the oe block, and
un-permutes rows/columns back to natural order.

Sharding: 16 alphas data-parallel over 8 cores (2 per core).
"""

import sys

sys.path.insert(0, "/opt/trn_rl_repo")

import numpy as np

N = 1024
H = 512  # half dimension (parity-reduced block size)
B = 16
NCORES = 8
APC = B // NCORES  # alphas per core
P = 128
KC = H // P  # contraction chunks (4)
MC = H // P  # output row chunks (4)

_cache = {}


def _build_module(reps=1):
    import contextlib

    import concourse.bacc as bacc
    import concourse.mybir as mybir
    import concourse.tile as tile

    f32 = mybir.dt.float32
    bf16 = mybir.dt.bfloat16

    nc = bacc.Bacc(
        "TRN2",
        target_bir_lowering=False,
        debug=False,
        num_devices=NCORES,
    )

    # A^T / B^T pre-permuted on host to [p, kc*512+q] = X^T[kc*128+p, q]
    # so each loads with a single contiguous-per-partition DMA.
    at_d = nc.dram_tensor("at", [P, KC * H], bf16, kind="ExternalInput")
    bt_d = nc.dram_tensor("bt", [P, KC * H], bf16, kind="ExternalInput")
    # Per-partition scalars: 2cos / -2sin of r*lam at p = kc*128+p.
    esc_d = nc.dram_tensor("esc", [P, APC * 2 * KC], f32, kind="ExternalInput")
    # Out: per (alpha, mb) a ragged slab of used width 2*(H-mb*P)+H:
    # [C_ee cols mb*128.., C_oo cols mb*128.., S' full row].
    out_d = nc.dram_tensor("out", [APC, MC, P, 3 * H], bf16, kind="ExternalOutput")

    with tile.TileContext(nc) as tc:
        with (
            tc.tile_pool(name="const", bufs=1) as cpool,
            tc.tile_pool(name="wts", bufs=3) as wpool,
            tc.tile_pool(name="outp", bufs=8) as outp,
            tc.tile_pool(name="psum", bufs=3, space="PSUM") as pp,
            tc.tile_pool(name="psums", bufs=2, space="PSUM") as pps,
        ):
            esc = cpool.tile([P, APC * 2 * KC], f32, name="esc")
            at = cpool.tile([P, KC * H], bf16, name="at")
            bt = cpool.tile([P, KC * H], bf16, name="bt")

            # Parallel queues: each DMA has ~1.5us issue+DGE latency, so
            # serializing them on one queue delays the first matmul.
            # Chunk-0 of at/bt loads separately so the first scalings and
            # matmuls start before the full halves land.
            nc.gpsimd.dma_start(esc[:], esc_d[:])
            nc.sync.dma_start(at[:, 0:H], at_d[:, 0:H])
            nc.scalar.dma_start(bt[:, 0:H], bt_d[:, 0:H])
            nc.sync.dma_start(at[:, H : KC * H], at_d[:, H : KC * H])
            nc.scalar.dma_start(bt[:, H : KC * H], bt_d[:, H : KC * H])

            rep_ctx = (
                tc.For_i(0, reps, 1) if reps > 1 else contextlib.nullcontext()
            )
            with rep_ctx:
                _emit_body(nc, tc, at, bt, esc, wpool, outp, pp, pps,
                           out_d, mybir)

    nc.compile()
    return nc


def _emit_body(nc, tc, at, bt, esc, wpool, outp, pp, pps, out_d, mybir):
    f32 = mybir.dt.float32
    bf16 = mybir.dt.bfloat16
    Act = mybir.ActivationFunctionType

    for a in range(APC):
        # --- per-alpha diagonal scalings (DVE, 2x bf16) -------------
        lac = wpool.tile([P, KC * H], bf16, tag="lac")
        lbs = wpool.tile([P, KC * H], bf16, tag="lbs")
        lbc = wpool.tile([P, KC * H], bf16, tag="lbc")
        for kc in range(KC):
            c_er = a * 2 * KC + kc
            c_ei = a * 2 * KC + KC + kc
            sl = slice(kc * H, (kc + 1) * H)
            nc.vector.tensor_scalar_mul(
                lac[:, sl], at[:, sl], esc[:, c_er : c_er + 1]
            )
            # lbs on ACT so lac/lbs scale in parallel (the kc-th ee+s
            # matmul pair needs both; ACT is idle during the scalings).
            nc.scalar.activation(
                lbs[:, sl], bt[:, sl], Act.Copy,
                scale=esc[:, c_ei : c_ei + 1],
            )
        for kc in range(KC):
            c_er = a * 2 * KC + kc
            sl = slice(kc * H, (kc + 1) * H)
            nc.vector.tensor_scalar_mul(
                lbc[:, sl], bt[:, sl], esc[:, c_er : c_er + 1]
            )

        # --- matmuls, one mb at a time (ee+s then oo) so each slab
        # ships as soon as its row-block is done -------------------
        # C_ee/C_oo symmetric: compute and ship only columns >= mb*128
        # (the host mirrors the lower-triangle blocks).  Slab layout
        # per (a, mb): [cee_fresh (fw) | coo_fresh (fw) | s (H)].
        for mb in range(MC):
            lo = mb * P
            fw = H - lo  # fresh width
            pee = pp.tile([P, H], f32, tag="pee")
            ps = pps.tile([P, H], f32, tag="ps")
            poo = pp.tile([P, H], f32, tag="poo")
            for kc in range(KC):
                wap = at[:, kc * H + mb * P : kc * H + (mb + 1) * P]
                st = kc == 0
                sp = kc == KC - 1
                nc.tensor.matmul(
                    pee[:, 0:fw], wap, lac[:, kc * H + lo : (kc + 1) * H],
                    start=st, stop=sp,
                )
                nc.tensor.matmul(
                    ps[:], wap, lbs[:, kc * H : (kc + 1) * H],
                    start=st, stop=sp,
                )
            stg = outp.tile([P, 3 * H], bf16, tag="stge", name=f"stge{mb}_{a}")
            nc.scalar.activation(stg[:, 0:fw], pee[:, 0:fw], Act.Copy)
            nc.vector.tensor_copy(stg[:, 2 * fw : 2 * fw + H], ps[:])
            for kc in range(KC):
                wap = bt[:, kc * H + mb * P : kc * H + (mb + 1) * P]
                st = kc == 0
                sp = kc == KC - 1
                nc.tensor.matmul(
                    poo[:, 0:fw], wap, lbc[:, kc * H + lo : (kc + 1) * H],
                    start=st, stop=sp,
                )
            nc.scalar.activation(stg[:, fw : 2 * fw], poo[:, 0:fw], Act.Copy)
            # Alternate HWDGE queues so consecutive slabs drain in parallel.
            dq = nc.sync if mb % 2 == 0 else nc.scalar
            dq.dma_start(
                out_d[a, mb, :, 0 : 2 * fw + H], stg[:, 0 : 2 * fw + H]
            )


def _get_module():
    if "nc" not in _cache:
        _cache["nc"] = _build_module()
    return _cache["nc"]


def _host_precompute(alpha_real, alpha_imag, evals):
    """Per-alpha scalars, mirroring the reference's fp32 arithmetic.

    Returns esc_all [B, 2, KC, P] f32 and the per-alpha phase bases w.
    """
    ar = np.asarray(alpha_real, np.float32)
    ai = np.asarray(alpha_imag, np.float32)
    ev = np.asarray(evals, np.float32)

    esc_all = np.empty((B, 2, KC, P), np.float32)
    ws = []

    for b in range(B):
        alpha = np.complex64(complex(ar[b], ai[b]))
        r = np.float32(np.abs(alpha)) + np.float32(1e-10)
        eit = np.complex64(alpha / r)
        w = np.complex128(1j) * np.complex128(eit)
        ws.append(w)

        t32 = (np.float32(r) * ev[:H]).astype(np.float32)
        t64 = t32.astype(np.float64)
        esc_all[b, 0] = (2.0 * np.cos(t64)).astype(np.float32).reshape(KC, P)
        esc_all[b, 1] = (-2.0 * np.sin(t64)).astype(np.float32).reshape(KC, P)

    return esc_all, ws


def _build_in_maps(alpha_real, alpha_imag, evals, evecs):
    import ml_dtypes

    bf = ml_dtypes.bfloat16

    evecs_f = np.asarray(evecs, np.float32)
    A = evecs_f[0::2, :H]  # even rows, negative-lambda columns
    Bm = evecs_f[1::2, :H]
    # [p, kc*H+q] = X^T[kc*128+p, q] so one DMA loads all four chunks
    at_np = np.ascontiguousarray(
        A.T.reshape(KC, P, H).transpose(1, 0, 2).reshape(P, KC * H).astype(bf)
    )
    bt_np = np.ascontiguousarray(
        Bm.T.reshape(KC, P, H).transpose(1, 0, 2).reshape(P, KC * H).astype(bf)
    )

    esc_all, ws = _host_precompute(alpha_real, alpha_imag, evals)

    in_maps = []
    for c in range(NCORES):
        bs = [c * APC + a for a in range(APC)]
        esc = np.empty((P, APC * 2 * KC), np.float32)
        for a, b in enumerate(bs):
            for which in range(2):
                cols = a * 2 * KC + which * KC
                esc[:, cols : cols + KC] = esc_all[b, which].T
        in_maps.append({"at": at_np, "bt": bt_np, "esc": esc})
    return in_maps, ws


def kernel(alpha_real, alpha_imag, evals, evecs):
    from concourse import bass_utils

    nc = _get_module()
    in_maps, ws = _build_in_maps(alpha_real, alpha_imag, evals, evecs)

    res = bass_utils.run_bass_kernel_spmd(
        nc, in_maps, core_ids=list(range(NCORES))
    )

    # Host: unpack ragged slabs, mirror the symmetric C lower triangle,
    # rank-1 Toeplitz phase application + parity un-permutation.
    rng = np.arange(N)
    out = np.empty((B, N, N), np.complex64)
    for c in range(NCORES):
        arr = np.asarray(res.results[c]["out"])  # [APC, MC, P, 3H] bf16
        for a in range(APC):
            b = c * APC + a
            cee = np.empty((H, H), np.float32)
            coo = np.empty((H, H), np.float32)
            sp = np.empty((H, H), np.float32)
            for mb in range(MC):
                lo = mb * P
                fw = H - lo
                slab = arr[a, mb].astype(np.float32)  # [P, 3H]
                cee[lo : lo + P, lo:H] = slab[:, 0:fw]
                coo[lo : lo + P, lo:H] = slab[:, fw : 2 * fw]
                sp[lo : lo + P, :] = slab[:, 2 * fw : 2 * fw + H]
            for mb in range(1, MC):
                lo = mb * P
                cee[lo : lo + P, 0:lo] = cee[0:lo, lo : lo + P].T
                coo[lo : lo + P, 0:lo] = coo[0:lo, lo : lo + P].T
            w = ws[b]
            u = w**rng  # u_i = w^i
            v = w ** (-rng)  # v_j = w^-j ; P_ij = u_i v_j
            pee = np.outer(u[0::2], v[0::2])
            poo = np.outer(u[1::2], v[1::2])
            peo = np.outer(u[0::2], v[1::2])
            poe = np.outer(u[1::2], v[0::2])

            outr = out.real[b]
            outi = out.imag[b]
            # even blocks: Da = P o C
            outr[0::2, 0::2] = (pee.real * cee).astype(np.float32)
            outi[0::2, 0::2] = (pee.imag * cee).astype(np.float32)
            outr[1::2, 1::2] = (poo.real * coo).astype(np.float32)
            outi[1::2, 1::2] = (poo.imag * coo).astype(np.float32)
            # odd blocks: S = -S'; Da = P o (-iS) = i P o S'
            outr[0::2, 1::2] = (-peo.imag * sp).astype(np.float32)
            outi[0::2, 1::2] = (peo.real * sp).astype(np.float32)
            spt = sp.T
            outr[1::2, 0::2] = (-poe.imag * spt).astype(np.float32)
            outi[1::2, 0::2] = (poe.real * spt).astype(np.float32)
    return out

